# revision 8
# baseline (speedup 1.0000x reference)
"""Trainium2 Bass kernel for nn_CliffordEquivariantGatingBlock.

Math (validated against the reference in numpy):
  Blades are handled in BITMASK order (grade order [0,1,2,4,3,5,6,7] maps to
  bitmask by swapping positions 3 and 4; the swap is folded into host-side
  weight construction and the device-side python index maps — zero cost).

  p  = x @ Wp / sqrt(2) (+bias on blade 0)         probe (pre-scaled)
  m' = x @ (delta * Wm) (+bias)                    match with delta folded in,
                                                   delta = (-1)^C(grade,2) = [+,+,+,-,+,-,-,-]
  m_raw = delta * m'  (un-folds delta -> raw match)
  b  = sum_k p_k * m_raw_k                         gate (= <beta(probe),match> scaled)
  q  = sum_k m'_k * m_raw_k                        (= sum delta * match^2)
  invn = exp(-0.25 * ln(q^2 + 1e-16))
  pp = p * invn * (1 - (b>0))
  r_j = sum_k chi_j(k) * pp_{k XOR j} * m'_k       geometric product, where
        chi_j(k) = (-1)^{<d_j,k>}; the character signs are realized as
        subtract at tree level 1 for j in {2,3,4,5}, level 2 for j in {4,5,6,7}.
  out = where(b>0, p, r)                           (already has the /sqrt2)

Layout: b on partitions. x host-transposed to [64 m, 8 k, B] so the PE consumes
it as the stationary operand: out[128 b, (k, p/m, n)] = x_chunk.T @ W.
Output slab [128 b, 256 cell, 8 j] DMAs contiguously to out[b, n, j].
"""

import math
import numpy as np

import concourse.bass as bass
import concourse.bacc as bacc
import concourse.mybir as mybir
import concourse.tile as tile
from concourse.bass_utils import run_bass_kernel_spmd

AluOp = mybir.AluOpType
AF = mybir.ActivationFunctionType
FP32 = mybir.dt.float32

N_CORES = 8
B_TOTAL = 262144
IN = 64
OUT = 64
BC = B_TOTAL // N_CORES          # per-core batch
NB = 512                         # b per slab
SUB = NB // 128                  # 128-b matmul subtiles per slab
G = SUB * OUT                    # cells (b x n pairs) per partition per slab

BITS = np.array([0b000, 0b001, 0b010, 0b100, 0b011, 0b101, 0b110, 0b111])
ORDER = np.argsort(BITS)         # grade position of bitmask i (self-inverse)
SUBSPACE = np.array([0, 1, 1, 1, 2, 2, 2, 3])
DELTA_BM = np.array([1, 1, 1, -1, 1, -1, -1, -1], dtype=np.float32)
EPS = 1e-16


def _host_weights(w, b, scale, fold_delta):
    """-> wt [64 m, 8 k_bitmask, 64 n] f32, bias [64 n] f32 (blade 0 only)."""
    w_full = np.asarray(w, dtype=np.float32)[:, :, SUBSPACE]      # (n, m, 8 grade)
    w_bm = w_full[:, :, ORDER] * np.float32(scale)                # (n, m, 8 bitmask)
    if fold_delta:
        w_bm = w_bm * DELTA_BM[None, None, :]
    wt = np.ascontiguousarray(w_bm.transpose(1, 2, 0), dtype=np.float32)
    return wt, (np.asarray(b, dtype=np.float32) * np.float32(scale))


def build_nc(bc=BC):
    nslab = bc // NB
    nc = bacc.Bacc()

    xt = nc.declare_dram_parameter("xt", [IN, 8, bc], FP32, isOutput=False)
    wt = nc.declare_dram_parameter("wt", [IN, 8, 128], FP32, isOutput=False)
    bias = nc.declare_dram_parameter("bias", [128, 129], FP32, isOutput=False)
    out_d = nc.declare_dram_parameter("out", [bc, 512], FP32, isOutput=True)

    out_v = out_d[:].rearrange("(i s p) c -> i s p c", i=nslab, s=SUB, p=128)

    with tile.TileContext(nc) as tc:
        with (
            tc.tile_pool(name="const", bufs=1) as constp,
            tc.tile_pool(name="xs", bufs=2) as xsp,
            tc.tile_pool(name="slab", bufs=2) as slabp,
            tc.tile_pool(name="work", bufs=2) as workp,
            tc.tile_pool(name="gate", bufs=2) as gatep,
            tc.tile_pool(name="outp", bufs=2) as outp,
            tc.tile_pool(name="psum", bufs=3, space="PSUM") as psum,
        ):
            w_sb = constp.tile([IN, 8, 128], FP32)
            bias_sb = constp.tile([128, 129], FP32)
            nc.sync.dma_start(out=w_sb[:], in_=wt[:])
            nc.sync.dma_start(out=bias_sb[:], in_=bias[:])

            for i in range(nslab):
                xs = xsp.tile([IN, 8, NB], FP32, tag="xs")
                nc.sync.dma_start(out=xs[:], in_=xt[:, :, i * NB:(i + 1) * NB])

                p_sb = slabp.tile([128, 8, G], FP32, tag="p")
                m_sb = slabp.tile([128, 8, G], FP32, tag="m")

                for s in range(SUB):
                    ps = psum.tile([128, 8, 128], FP32, tag="ps")
                    for k in range(8):
                        # grade-ordered DRAM x; bitmask blade k lives at ORDER[k]
                        nc.tensor.matmul(
                            ps[:, k, :],
                            xs[:, ORDER[k], s * 128:(s + 1) * 128],
                            w_sb[:, k, :],
                        )
                    # evacuate PSUM -> SBUF slabs on ScalarE (ACT)
                    nc.scalar.copy(p_sb[:, :, s * OUT:(s + 1) * OUT], ps[:, :, 0:OUT])
                    nc.scalar.copy(m_sb[:, :, s * OUT:(s + 1) * OUT], ps[:, :, OUT:128])

                # bias on blade 0 (broadcast over partitions=b and subtiles)
                bp_ap = bias_sb[:, 0:OUT].unsqueeze(1).broadcast_to([128, SUB, OUT])
                bm_ap = bias_sb[:, OUT:128].unsqueeze(1).broadcast_to([128, SUB, OUT])
                p0 = p_sb[:, 0, :].rearrange("p (s n) -> p s n", s=SUB)
                m0 = m_sb[:, 0, :].rearrange("p (s n) -> p s n", s=SUB)
                nc.vector.tensor_add(p0, p0, bp_ap)
                nc.vector.tensor_add(m0, m0, bm_ap)

                # m_raw = delta * m'  (negate blades 3,5,6,7)
                m_raw = slabp.tile([128, 8, G], FP32, tag="mraw")
                nc.vector.tensor_copy(m_raw[:, 0:3, :], m_sb[:, 0:3, :])
                nc.vector.tensor_copy(m_raw[:, 4, :], m_sb[:, 4, :])
                nc.vector.tensor_scalar_mul(m_raw[:, 3, :], m_sb[:, 3, :], -1.0)
                nc.vector.tensor_scalar_mul(m_raw[:, 5:8, :], m_sb[:, 5:8, :], -1.0)

                def tree(prod_t, out_ap, sub1, sub2):
                    t1 = workp.tile([128, 4, G], FP32, tag="t1")
                    t2 = workp.tile([128, 2, G], FP32, tag="t2")
                    op1 = nc.vector.tensor_sub if sub1 else nc.vector.tensor_add
                    op2 = nc.vector.tensor_sub if sub2 else nc.vector.tensor_add
                    op1(t1[:], prod_t[:, 0:8:2, :], prod_t[:, 1:8:2, :])
                    op2(t2[:], t1[:, 0:4:2, :], t1[:, 1:4:2, :])
                    nc.vector.tensor_add(out_ap, t2[:, 0, :], t2[:, 1, :])

                prod = workp.tile([128, 8, G], FP32, tag="prod")
                b_g = gatep.tile([128, G], FP32, tag="bg")
                q_g = gatep.tile([128, G], FP32, tag="qg")

                nc.vector.tensor_mul(prod[:], p_sb[:], m_raw[:])
                tree(prod, b_g[:], False, False)
                prod = workp.tile([128, 8, G], FP32, tag="prod")
                nc.vector.tensor_mul(prod[:], m_sb[:], m_raw[:])
                tree(prod, q_g[:], False, False)

                # invn = exp(-0.25*ln(q^2+eps)) on ACT
                sq = gatep.tile([128, G], FP32, tag="sq")
                invn = gatep.tile([128, G], FP32, tag="invn")
                nc.scalar.activation(sq[:], q_g[:], AF.Square)
                nc.scalar.activation(invn[:], sq[:], AF.Ln, bias=bias_sb[:, 128:129])
                nc.scalar.activation(invn[:], invn[:], AF.Exp, scale=-0.25)

                mask = gatep.tile([128, G], mybir.dt.uint8, tag="mask")
                nmask = gatep.tile([128, G], FP32, tag="nmask")
                ws = gatep.tile([128, G], FP32, tag="ws")
                nc.vector.tensor_scalar(mask[:], b_g[:], 0.0, None, op0=AluOp.is_gt)
                nc.vector.tensor_scalar(nmask[:], b_g[:], 0.0, None, op0=AluOp.is_le)
                nc.vector.tensor_mul(ws[:], invn[:], nmask[:])

                # pp = p * ws (broadcast over blades)
                pp = slabp.tile([128, 8, G], FP32, tag="pp")
                ws_b = ws[:].unsqueeze(1).broadcast_to([128, 8, G])
                nc.vector.tensor_mul(pp[:], p_sb[:], ws_b)

                out_sb = outp.tile([128, G, 8], FP32, tag="out")
                ppv = pp[:].rearrange("p (a b c) g -> p a b c g", a=2, b=2, c=2)

                for j in range(8):
                    pj = ppv
                    if j & 4:
                        pj = pj[:, ::-1]
                    if j & 2:
                        pj = pj[:, :, ::-1]
                    if j & 1:
                        pj = pj[:, :, :, ::-1]
                    prod = workp.tile([128, 8, G], FP32, tag="prod")
                    if j in (2, 5):
                        # stride pattern (+,-,+)/(-,+,-) cannot merge to <=3 free
                        # dims; split along the outer xor dim.
                        for aa in range(2):
                            nc.vector.tensor_mul(
                                prod[:, 4 * aa:4 * aa + 4, :], pj[:, aa],
                                m_sb[:, 4 * aa:4 * aa + 4, :])
                    else:
                        # merge adjacent same-flip dims to <=3 free dims
                        if j in (0, 7):
                            pj3 = pp[:] if j == 0 else pp[:, ::-1, :]
                        elif j in (1, 6):
                            v = pp[:].rearrange("p (u c) g -> p u c g", c=2)
                            pj3 = v[:, :, ::-1] if j == 1 else v[:, ::-1]
                        else:  # 3, 4
                            v = pp[:].rearrange("p (a u) g -> p a u g", a=2)
                            pj3 = v[:, :, ::-1] if j == 3 else v[:, ::-1]
                        nc.vector.tensor_mul(prod[:], pj3, m_sb[:])
                    jpos = 7 - j if j in (3, 4) else j   # bitmask -> grade position
                    tree(prod, out_sb[:, :, jpos], j in (2, 3, 4, 5), j in (4, 5, 6, 7))

                # select: where b>0 overwrite with p  (k order per grade position)
                mb = mask[:].unsqueeze(2)
                nc.vector.copy_predicated(
                    out_sb[:, :, 0:3], mb.broadcast_to([128, G, 3]),
                    p_sb[:, 0:3, :].rearrange("p k g -> p g k"))
                nc.vector.copy_predicated(
                    out_sb[:, :, 3:5], mb.broadcast_to([128, G, 2]),
                    p_sb[:, 4:2:-1, :].rearrange("p k g -> p g k"))
                nc.vector.copy_predicated(
                    out_sb[:, :, 5:8], mb.broadcast_to([128, G, 3]),
                    p_sb[:, 5:8, :].rearrange("p k g -> p g k"))

                src = out_sb[:].rearrange("p (s n) j -> p s (n j)", s=SUB)
                dst = out_v[i].rearrange("s p c -> p s c")
                nc.sync.dma_start(out=dst, in_=src)

    nc.finalize()
    return nc


_NC_CACHE = {}


def _get_nc(bc):
    if bc not in _NC_CACHE:
        _NC_CACHE[bc] = build_nc(bc)
    return _NC_CACHE[bc]


def kernel(x, w_probe, b_probe, w_match, b_match, _trace=False):
    x = np.asarray(x, dtype=np.float32)
    B = x.shape[0]
    n_cores = N_CORES
    bc = B // n_cores
    assert bc % NB == 0, f"per-core batch {bc} not divisible by {NB}"

    wp, bp = _host_weights(w_probe, b_probe, 1.0 / math.sqrt(2.0), False)
    wm, bm = _host_weights(w_match, b_match, 1.0, True)
    wt = np.concatenate([wp, wm], axis=2)              # [64, 8, 128]
    bias = np.broadcast_to(
        np.concatenate([bp, bm, [np.float32(EPS)]])[None, :], (128, 129)).copy().astype(np.float32)

    nc = _get_nc(bc)

    in_maps = []
    for c in range(n_cores):
        xs = x[c * bc:(c + 1) * bc]                    # (bc, 64, 8) grade order
        xt = np.ascontiguousarray(xs.transpose(1, 2, 0))   # (64, 8, bc)
        in_maps.append({"xt": xt, "wt": wt, "bias": bias})

    try:
        res = run_bass_kernel_spmd(nc, in_maps, list(range(n_cores)), trace=_trace)
    except ModuleNotFoundError:
        res = run_bass_kernel_spmd(nc, in_maps, list(range(n_cores)), trace=False)
    outs = [np.asarray(res.results[c]["out"]).reshape(bc, OUT, 8) for c in range(n_cores)]
    out = np.concatenate(outs, axis=0)
    kernel.last_exec_time_ns = res.exec_time_ns
    kernel.last_profile = res.profile_json
    return out
